# revision 7
# baseline (speedup 1.0000x reference)
"""Haar DWT (2x2 stride-2 depthwise conv, fixed +-0.5 weights) on 8 trn2 cores.

Input  x: (8, 128, 512, 512) f32.
Output: tuple (hh, hl, lh, ll), each (8, 128, 256, 256) f32.

Sharding: pure data parallel over the batch dim — core b processes x[b].
Per-core layout: channel dim (128) -> SBUF partitions; tile over image rows.

Dataflow per tile of R rows:
  DMA in  -> ACT: x *= 0.5 (in place) -> DVE: S/D = even_rows +/- odd_rows
  -> DVE: band = S_e +/- S_o, D_e +/- D_o -> DMA out (4 bands).
"""

import numpy as np

N_CORES = 8
C = 128  # channels == SBUF partitions
H = 512
W = 512

BANDS = ("hh", "hl", "lh", "ll")  # reference return order

_CACHE = {}

# test.py can flip these before calling kernel()
TRACE = False
LAST_RESULTS = None


def _build(h, w, rows_per_tile, x_bufs=4, sd_bufs=2):
    import concourse.bacc as bacc
    import concourse.tile as tile
    import concourse.mybir as mybir

    f32 = mybir.dt.float32
    nc = bacc.Bacc("TRN2", target_bir_lowering=False, debug=False,
                   num_devices=N_CORES)

    x = nc.dram_tensor("x", [C, h, w], f32, kind="ExternalInput").ap()
    outs = {
        name: nc.dram_tensor(name, [C, h // 2, w // 2], f32,
                             kind="ExternalOutput").ap()
        for name in BANDS
    }

    R = rows_per_tile
    assert h % R == 0 and R % 4 == 0

    with tile.TileContext(nc) as tc:
        with (
            tc.tile_pool(name="xp", bufs=x_bufs) as xp,
            tc.tile_pool(name="sd", bufs=sd_bufs) as sd,
        ):
            for r0 in range(0, h, R):
                xt = xp.tile([C, R, w], f32)
                # Split the load into 4-row sub-DMAs: 8 KiB-per-partition
                # packets run ~2x faster per byte than 32 KiB ones, and
                # back-to-back issue into one tile avoids slot stalls.
                for k in range(0, R, 4):
                    nc.sync.dma_start(out=xt[:, k:k + 4, :],
                                      in_=x[:, r0 + k:r0 + k + 4, :])
                nc.scalar.mul(xt[:], xt[:], 0.5)

                ev = xt[:, 0::2, :]
                od = xt[:, 1::2, :]
                S = sd.tile([C, R // 2, w], f32, tag="S")
                D = sd.tile([C, R // 2, w], f32, tag="D")
                nc.vector.tensor_add(out=S[:], in0=ev, in1=od)
                nc.vector.tensor_sub(out=D[:], in0=ev, in1=od)

                # Bands overwrite the (fully consumed) x tile — saves a pool,
                # letting everything triple-buffer within SBUF. Each band gets
                # a flat contiguous quarter of the tile so its store DMA emits
                # one contiguous per-partition descriptor.
                xf = xt.rearrange("p r w -> p (r w)")
                q = (R // 2) * (w // 2)
                slots = {
                    name: xf[:, i * q:(i + 1) * q].rearrange(
                        "p (r c) -> p r c", c=w // 2)
                    for i, name in enumerate(BANDS)
                }
                pairs = {
                    "ll": (S, "add"), "lh": (S, "sub"),
                    "hl": (D, "add"), "hh": (D, "sub"),
                }
                for name in BANDS:
                    src, op = pairs[name]
                    bt = slots[name]
                    e = src[:, :, 0::2]
                    o = src[:, :, 1::2]
                    if op == "add":
                        nc.vector.tensor_add(out=bt, in0=e, in1=o)
                    else:
                        nc.vector.tensor_sub(out=bt, in0=e, in1=o)
                    nc.sync.dma_start(out=outs[name][:, r0 // 2:(r0 + R) // 2, :],
                                      in_=bt)
    nc.compile()
    return nc


def _get_nc():
    key = (H, W)
    if key not in _CACHE:
        _CACHE[key] = _build(H, W, rows_per_tile=16)
    return _CACHE[key]


def kernel(x: np.ndarray):
    global LAST_RESULTS
    from concourse.bass_utils import run_bass_kernel_spmd

    assert x.shape == (N_CORES, C, H, W), x.shape
    x = np.ascontiguousarray(x, dtype=np.float32)

    nc = _get_nc()
    in_maps = [{"x": x[b]} for b in range(N_CORES)]
    res = run_bass_kernel_spmd(nc, in_maps, core_ids=list(range(N_CORES)),
                               trace=TRACE)
    LAST_RESULTS = res

    out = tuple(
        np.stack([res.results[b][name] for b in range(N_CORES)])
        for name in BANDS
    )
    return out


# revision 9
# speedup vs baseline: 1.0586x; 1.0586x over previous
"""Haar DWT (2x2 stride-2 depthwise conv, fixed +-0.5 weights) on 8 trn2 cores.

Input  x: (8, 128, 512, 512) f32.
Output: tuple (hh, hl, lh, ll), each (8, 128, 256, 256) f32.

Sharding: pure data parallel over the batch dim — core b processes x[b].
Per-core layout: channel dim (128) -> SBUF partitions; tile over image rows.

Dataflow per tile of R rows:
  DMA in  -> ACT: x *= 0.5 (in place) -> DVE: S/D = even_rows +/- odd_rows
  -> DVE: band = S_e +/- S_o, D_e +/- D_o -> DMA out (4 bands).
"""

import numpy as np

N_CORES = 8
C = 128  # channels == SBUF partitions
H = 512
W = 512

BANDS = ("hh", "hl", "lh", "ll")  # reference return order

_CACHE = {}

# test.py can flip these before calling kernel()
TRACE = False
LAST_RESULTS = None


def _build(h, w, rows_per_tile, x_bufs=5, sd_bufs=1):
    import concourse.bacc as bacc
    import concourse.tile as tile
    import concourse.mybir as mybir

    f32 = mybir.dt.float32
    nc = bacc.Bacc("TRN2", target_bir_lowering=False, debug=False,
                   num_devices=N_CORES)

    x = nc.dram_tensor("x", [C, h, w], f32, kind="ExternalInput").ap()
    outs = {
        name: nc.dram_tensor(name, [C, h // 2, w // 2], f32,
                             kind="ExternalOutput").ap()
        for name in BANDS
    }

    R = rows_per_tile
    assert h % R == 0 and R % 4 == 0

    with tile.TileContext(nc) as tc:
        with (
            tc.tile_pool(name="xp", bufs=x_bufs) as xp,
            tc.tile_pool(name="sd", bufs=sd_bufs) as sd,
        ):
            for r0 in range(0, h, R):
                xt = xp.tile([C, R, w], f32)
                # Split the load into 4-row sub-DMAs: 8 KiB-per-partition
                # packets run ~2x faster per byte than 32 KiB ones, and
                # back-to-back issue into one tile avoids slot stalls.
                for k in range(0, R, 4):
                    nc.sync.dma_start(out=xt[:, k:k + 4, :],
                                      in_=x[:, r0 + k:r0 + k + 4, :])
                nc.scalar.mul(xt[:], xt[:], 0.5)

                ev = xt[:, 0::2, :]
                od = xt[:, 1::2, :]
                S = sd.tile([C, R // 2, w], f32, tag="S")
                D = sd.tile([C, R // 2, w], f32, tag="D")
                nc.vector.tensor_add(out=S[:], in0=ev, in1=od)
                nc.vector.tensor_sub(out=D[:], in0=ev, in1=od)

                # Bands overwrite the (fully consumed) x tile — saves a pool,
                # letting everything triple-buffer within SBUF. Each band gets
                # a flat contiguous quarter of the tile so its store DMA emits
                # one contiguous per-partition descriptor.
                xf = xt.rearrange("p r w -> p (r w)")
                q = (R // 2) * (w // 2)
                slots = {
                    name: xf[:, i * q:(i + 1) * q].rearrange(
                        "p (r c) -> p r c", c=w // 2)
                    for i, name in enumerate(BANDS)
                }
                pairs = {
                    "ll": (S, "add"), "lh": (S, "sub"),
                    "hl": (D, "add"), "hh": (D, "sub"),
                }
                for name in BANDS:
                    src, op = pairs[name]
                    bt = slots[name]
                    e = src[:, :, 0::2]
                    o = src[:, :, 1::2]
                    if op == "add":
                        nc.vector.tensor_add(out=bt, in0=e, in1=o)
                    else:
                        nc.vector.tensor_sub(out=bt, in0=e, in1=o)
                    nc.sync.dma_start(out=outs[name][:, r0 // 2:(r0 + R) // 2, :],
                                      in_=bt)
    nc.compile()
    return nc


def _get_nc():
    key = (H, W)
    if key not in _CACHE:
        _CACHE[key] = _build(H, W, rows_per_tile=16)
    return _CACHE[key]


def kernel(x: np.ndarray):
    global LAST_RESULTS
    from concourse.bass_utils import run_bass_kernel_spmd

    assert x.shape == (N_CORES, C, H, W), x.shape
    x = np.ascontiguousarray(x, dtype=np.float32)

    nc = _get_nc()
    in_maps = [{"x": x[b]} for b in range(N_CORES)]
    res = run_bass_kernel_spmd(nc, in_maps, core_ids=list(range(N_CORES)),
                               trace=TRACE)
    LAST_RESULTS = res

    out = tuple(
        np.stack([res.results[b][name] for b in range(N_CORES)])
        for name in BANDS
    )
    return out


# revision 11
# speedup vs baseline: 1.0881x; 1.0279x over previous
"""Haar DWT (2x2 stride-2 depthwise conv, fixed +-0.5 weights) on 8 trn2 cores.

Input  x: (8, 128, 512, 512) f32.
Output: tuple (hh, hl, lh, ll), each (8, 128, 256, 256) f32.

Sharding: pure data parallel over the batch dim — core b processes x[b].
Per-core layout: channel dim (128) -> SBUF partitions; tile over image rows.

Dataflow per tile of R rows:
  DMA in  -> ACT: x *= 0.5 (in place) -> DVE: S/D = even_rows +/- odd_rows
  -> DVE: band = S_e +/- S_o, D_e +/- D_o -> DMA out (4 bands).
"""

import numpy as np

N_CORES = 8
C = 128  # channels == SBUF partitions
H = 512
W = 512

BANDS = ("hh", "hl", "lh", "ll")  # reference return order

_CACHE = {}

# test.py can flip these before calling kernel()
TRACE = False
LAST_RESULTS = None


def _build(h, w, rows_per_tile, x_bufs=5, sd_bufs=1):
    import concourse.bacc as bacc
    import concourse.tile as tile
    import concourse.mybir as mybir

    f32 = mybir.dt.float32
    nc = bacc.Bacc("TRN2", target_bir_lowering=False, debug=False,
                   num_devices=N_CORES)

    x = nc.dram_tensor("x", [C, h, w], f32, kind="ExternalInput").ap()
    outs = {
        name: nc.dram_tensor(name, [C, h // 2, w // 2], f32,
                             kind="ExternalOutput").ap()
        for name in BANDS
    }

    R = rows_per_tile
    assert h % R == 0 and R % 4 == 0

    with tile.TileContext(nc) as tc:
        with (
            tc.tile_pool(name="xp", bufs=x_bufs) as xp,
            tc.tile_pool(name="sd", bufs=sd_bufs) as sd,
        ):
            def emit_tile(r0, rt):
                xt = xp.tile([C, rt, w], f32, tag="xt")
                # Split the load into 4-row sub-DMAs: 8 KiB-per-partition
                # packets run ~2x faster per byte than 32 KiB ones, and
                # back-to-back issue into one tile avoids slot stalls.
                # Halve each chunk on ACT as soon as its sub-load lands.
                for k in range(0, rt, 4):
                    nc.sync.dma_start(out=xt[:, k:k + 4, :],
                                      in_=x[:, r0 + k:r0 + k + 4, :])
                    nc.scalar.mul(xt[:, k:k + 4, :], xt[:, k:k + 4, :], 0.5)

                S = sd.tile([C, rt // 2, w], f32, tag="S")
                D = sd.tile([C, rt // 2, w], f32, tag="D")

                # Bands overwrite the (fully consumed) x tile — saves a pool,
                # deepening x buffering. Each band gets a flat contiguous
                # quarter of the tile so its store DMA emits one contiguous
                # per-partition descriptor.
                xf = xt.rearrange("p r w -> p (r w)")
                q = (rt // 2) * (w // 2)
                slots = {
                    name: xf[:, i * q:(i + 1) * q].rearrange(
                        "p (r c) -> p r c", c=w // 2)
                    for i, name in enumerate(BANDS)
                }
                pairs = {
                    "ll": (S, "add"), "lh": (S, "sub"),
                    "hl": (D, "add"), "hh": (D, "sub"),
                }

                # Stage 1 in row-halves so the first half starts right after
                # its ACT chunks, overlapping the second half's sub-loads.
                # Both halves MUST finish before stage 2: its band outputs
                # overwrite xt rows that stage-1 half 1 still reads.
                n_half = 2 if rt >= 8 else 1
                hr = rt // n_half        # xt rows per half
                for hf in range(n_half):
                    ev = xt[:, hf * hr:(hf + 1) * hr:2, :]
                    od = xt[:, hf * hr + 1:(hf + 1) * hr:2, :]
                    Sh = S[:, hf * (hr // 2):(hf + 1) * (hr // 2), :]
                    Dh = D[:, hf * (hr // 2):(hf + 1) * (hr // 2), :]
                    nc.vector.tensor_add(out=Sh, in0=ev, in1=od)
                    nc.vector.tensor_sub(out=Dh, in0=ev, in1=od)
                # Stage 2 + store per band, so each band's store issues while
                # the next band computes.
                for name in BANDS:
                    src, op = pairs[name]
                    bt = slots[name]
                    e = src[:, :, 0::2]
                    o = src[:, :, 1::2]
                    if op == "add":
                        nc.vector.tensor_add(out=bt, in0=e, in1=o)
                    else:
                        nc.vector.tensor_sub(out=bt, in0=e, in1=o)
                    nc.sync.dma_start(out=outs[name][:, r0 // 2:(r0 + rt) // 2, :],
                                      in_=bt)

            # Main tiles, with the last tile tapered into R=4 minis to
            # shorten the final serial (load->ACT->DVE->store) chain.
            for r0 in range(0, h - R, R):
                emit_tile(r0, R)
            for r0 in range(h - R, h, 4):
                emit_tile(r0, 4)
    nc.compile()
    return nc


def _get_nc():
    key = (H, W)
    if key not in _CACHE:
        _CACHE[key] = _build(H, W, rows_per_tile=16)
    return _CACHE[key]


def kernel(x: np.ndarray):
    global LAST_RESULTS
    from concourse.bass_utils import run_bass_kernel_spmd

    assert x.shape == (N_CORES, C, H, W), x.shape
    x = np.ascontiguousarray(x, dtype=np.float32)

    nc = _get_nc()
    in_maps = [{"x": x[b]} for b in range(N_CORES)]
    res = run_bass_kernel_spmd(nc, in_maps, core_ids=list(range(N_CORES)),
                               trace=TRACE)
    LAST_RESULTS = res

    out = tuple(
        np.stack([res.results[b][name] for b in range(N_CORES)])
        for name in BANDS
    )
    return out


# revision 13
# speedup vs baseline: 1.1475x; 1.0546x over previous
"""Haar DWT (2x2 stride-2 depthwise conv, fixed +-0.5 weights) on 8 trn2 cores.

Input  x: (8, 128, 512, 512) f32.
Output: tuple (hh, hl, lh, ll), each (8, 128, 256, 256) f32.

Sharding: pure data parallel over the batch dim — core b processes x[b].
Per-core layout: channel dim (128) -> SBUF partitions; tile over image rows.

Dataflow per tile of R rows:
  DMA in  -> ACT: x *= 0.5 (in place) -> DVE: S/D = even_rows +/- odd_rows
  -> DVE: band = S_e +/- S_o, D_e +/- D_o -> DMA out (4 bands).
"""

import numpy as np

N_CORES = 8
C = 128  # channels == SBUF partitions
H = 512
W = 512

BANDS = ("hh", "hl", "lh", "ll")  # reference return order

_CACHE = {}

# test.py can flip these before calling kernel()
TRACE = False
LAST_RESULTS = None


def _build(h, w, rows_per_tile, x_bufs=5, sd_bufs=1):
    import concourse.bacc as bacc
    import concourse.tile as tile
    import concourse.mybir as mybir

    f32 = mybir.dt.float32
    nc = bacc.Bacc("TRN2", target_bir_lowering=False, debug=False,
                   num_devices=N_CORES)

    x = nc.dram_tensor("x", [C, h, w], f32, kind="ExternalInput").ap()
    outs = {
        name: nc.dram_tensor(name, [C, h // 2, w // 2], f32,
                             kind="ExternalOutput").ap()
        for name in BANDS
    }

    R = rows_per_tile
    assert h % R == 0 and R % 4 == 0

    with tile.TileContext(nc) as tc:
        with (
            tc.tile_pool(name="xp", bufs=x_bufs) as xp,
            tc.tile_pool(name="sd", bufs=sd_bufs) as sd,
        ):
            def emit_tile(r0, rt):
                xt = xp.tile([C, rt, w], f32, tag="xt")
                # Split the load into 4-row sub-DMAs: 8 KiB-per-partition
                # packets run ~2x faster per byte than 32 KiB ones, and
                # back-to-back issue into one tile avoids slot stalls.
                # Halve each chunk on ACT as soon as its sub-load lands.
                for k in range(0, rt, 4):
                    nc.sync.dma_start(out=xt[:, k:k + 4, :],
                                      in_=x[:, r0 + k:r0 + k + 4, :])
                    nc.scalar.mul(xt[:, k:k + 4, :], xt[:, k:k + 4, :], 0.5)

                S = sd.tile([C, rt // 2, w], f32, tag="S")
                D = sd.tile([C, rt // 2, w], f32, tag="D")

                # Bands overwrite the (fully consumed) x tile — saves a pool,
                # deepening x buffering. Each band gets a flat contiguous
                # quarter of the tile so its store DMA emits one contiguous
                # per-partition descriptor.
                xf = xt.rearrange("p r w -> p (r w)")
                q = (rt // 2) * (w // 2)
                slots = {
                    name: xf[:, i * q:(i + 1) * q].rearrange(
                        "p (r c) -> p r c", c=w // 2)
                    for i, name in enumerate(BANDS)
                }
                pairs = {
                    "ll": (S, "add"), "lh": (S, "sub"),
                    "hl": (D, "add"), "hh": (D, "sub"),
                }

                # Stage 1 in row-halves so the first half starts right after
                # its ACT chunks, overlapping the second half's sub-loads.
                # Both halves MUST finish before stage 2: its band outputs
                # overwrite xt rows that stage-1 half 1 still reads.
                n_half = 2 if rt >= 8 else 1
                hr = rt // n_half        # xt rows per half
                for hf in range(n_half):
                    ev = xt[:, hf * hr:(hf + 1) * hr:2, :]
                    od = xt[:, hf * hr + 1:(hf + 1) * hr:2, :]
                    Sh = S[:, hf * (hr // 2):(hf + 1) * (hr // 2), :]
                    Dh = D[:, hf * (hr // 2):(hf + 1) * (hr // 2), :]
                    nc.vector.tensor_add(out=Sh, in0=ev, in1=od)
                    nc.vector.tensor_sub(out=Dh, in0=ev, in1=od)
                # Stage 2 per band; stores are returned (deferred one tile by
                # the caller) so the NEXT tile's loads sit ahead of this
                # tile's compute-gated stores in the sequencer's program
                # order — avoids head-of-line blocking of load issue.
                stores = []
                for name in BANDS:
                    src, op = pairs[name]
                    bt = slots[name]
                    e = src[:, :, 0::2]
                    o = src[:, :, 1::2]
                    if op == "add":
                        nc.vector.tensor_add(out=bt, in0=e, in1=o)
                    else:
                        nc.vector.tensor_sub(out=bt, in0=e, in1=o)
                    stores.append((outs[name][:, r0 // 2:(r0 + rt) // 2, :], bt))
                return stores

            # Main tiles, with the last tile tapered into R=4 minis to
            # shorten the final serial (load->ACT->DVE->store) chain.
            # Stores trail by one tile (see emit_tile).
            pending = []
            for r0 in range(0, h - R, R):
                nxt = emit_tile(r0, R)
                for dst, src in pending:
                    nc.sync.dma_start(out=dst, in_=src)
                pending = nxt
            for r0 in range(h - R, h, 4):
                nxt = emit_tile(r0, 4)
                for dst, src in pending:
                    nc.sync.dma_start(out=dst, in_=src)
                pending = nxt
            for dst, src in pending:
                nc.sync.dma_start(out=dst, in_=src)
    nc.compile()
    return nc


def _get_nc():
    key = (H, W)
    if key not in _CACHE:
        _CACHE[key] = _build(H, W, rows_per_tile=16)
    return _CACHE[key]


def kernel(x: np.ndarray):
    global LAST_RESULTS
    from concourse.bass_utils import run_bass_kernel_spmd

    assert x.shape == (N_CORES, C, H, W), x.shape
    x = np.ascontiguousarray(x, dtype=np.float32)

    nc = _get_nc()
    in_maps = [{"x": x[b]} for b in range(N_CORES)]
    res = run_bass_kernel_spmd(nc, in_maps, core_ids=list(range(N_CORES)),
                               trace=TRACE)
    LAST_RESULTS = res

    out = tuple(
        np.stack([res.results[b][name] for b in range(N_CORES)])
        for name in BANDS
    )
    return out
